# revision 13
# baseline (speedup 1.0000x reference)
"""Dot-product attention TRN2 Bass kernel (v5: bf16, row-tiled QK, DMA-xbar
input transposes, paired-FD exp).

Full inputs: queries/keys/values [32, 2048, 64] fp32.
Sharding: 32 heads split across 8 NeuronCores (4 heads each), no communication.

Per-core schedule (heads processed as 2 pairs; all matmul data bf16, fp32 PSUM):
  1. SWDGE cast-DMAs load Q/K/V per pair as bf16 into [128q, 16t, (2h x 64d)];
     DMA-xbar transposes (HWDGE, SBUF->SBUF bf16) then build Q^T/K^T
     [128(=2h x 64d), 2048q] with zero PE/DVE involvement. The whole
     preprocessing for step i+1 is prefetched at the start of step i.
  2. V|ones staged per head as [128k, 16t, 68] (col 64 = ones -> softmax
     denominator comes free out of the AV matmul).
  3. One flat unit stream over (step=rep x pair, q-chunk of 512, k-tile),
     global unit index u, scores in a static PSUM tile [128, 3, 1024]
     (6 banks), slot = u % 3:
       S^T halves = row-tiled concurrent matmul pair (head A rows 0-63, head B
       rows 64-127) -> score slot.
       exp(S^T/8): at u%3==1 ONE ACTIVATE FD=2048 covers slots 0-1 (two
       units); at u%3==2 an FD=1024 ACTIVATE covers slot 2. Fewer, bigger
       ACTIVATEs amortize the ~220ns per-op overhead of the bottleneck engine.
       AV: O[128q, 65] += P^T-slice (stationary) @ V|ones, 8 small matmuls,
       emitted ~2 units behind ACT via a global pending queue.
  4. Normalize straight from PSUM (reciprocal of denominator column +
     broadcast multiply) into fp32 staging; one DMA out per head.
PSUM budget: scores 6 banks + accumulators 2x1 = 8.
No max-subtraction: scores ~N(0,1), exp safe in fp32.
"""
import sys

sys.path.insert(0, "/opt/trn_rl_repo")

from contextlib import ExitStack

import numpy as np

import concourse.bass as bass
import concourse.tile as tile
from concourse import bacc, mybir
from concourse.bass_utils import run_bass_kernel_spmd

F32 = mybir.dt.float32
BF16 = mybir.dt.bfloat16
AF = mybir.ActivationFunctionType

N_CORES = 8
H = 4  # heads per core
NP = 2  # head pairs per core
L = 2048
D = 64
NT = L // 128  # 16 k/q tiles of 128
QC = 512  # q-chunk
NQC = L // QC  # 4
SCALE = 1.0 / 8.0  # 1/sqrt(64)

_NC_CACHE = None


def _build_nc(reps=1):
    nc = bacc.Bacc("TRN2", target_bir_lowering=False, debug=False)
    q_d = nc.dram_tensor("queries", [H, L, D], F32, kind="ExternalInput").ap()
    k_d = nc.dram_tensor("keys", [H, L, D], F32, kind="ExternalInput").ap()
    v_d = nc.dram_tensor("values", [H, L, D], F32, kind="ExternalInput").ap()
    o_d = nc.dram_tensor("out", [H, L, D], F32, kind="ExternalOutput").ap()

    with tile.TileContext(nc) as tc, ExitStack() as ctx:
        stage = ctx.enter_context(tc.tile_pool(name="stage", bufs=2))
        tpose = ctx.enter_context(tc.tile_pool(name="tpose", bufs=2))
        vpool = ctx.enter_context(tc.tile_pool(name="vpool", bufs=3))
        ptbp = ctx.enter_context(tc.tile_pool(name="ptbp", bufs=2))
        ptsp = ctx.enter_context(tc.tile_pool(name="ptsp", bufs=2))
        outp = ctx.enter_context(tc.tile_pool(name="outp", bufs=3))
        rcp = ctx.enter_context(tc.tile_pool(name="rcp", bufs=4))
        ssp = ctx.enter_context(tc.tile_pool(name="ssp", bufs=1, space="PSUM"))
        acc_ = ctx.enter_context(tc.tile_pool(name="acc", bufs=1, space="PSUM"))

        # static score buffer: 3 slots of [128, 1024] fp32 (2 banks each)
        ss = ssp.tile([128, 3, 1024], F32)

        class Step:
            def __init__(self, p):
                self.p = p
                self.qt2 = None
                self.kt2 = None
                self.vo = None
                self.os_h = None
                self.accs = [None, None]

        def preproc(st):
            """Load + preprocess one step: cast-DMAs in, xbar-transpose DMAs
            to build Q^T/K^T, DVE copy for V|ones. All DMA/DVE — rides under
            the previous step's ACT-bound main loop."""
            p = st.p
            qs = stage.tile([128, NT, 128], BF16, tag="qstg", name="qs")
            ks = stage.tile([128, NT, 128], BF16, tag="kstg", name="ks")
            vs = stage.tile([128, NT, 128], BF16, tag="vstg", name="vs")
            for h in range(2):
                src = lambda t_d: t_d[2 * p + h].rearrange(
                    "(t q) d -> q t d", q=128
                )
                nc.gpsimd.dma_start(ks[:, :, h * 64 : (h + 1) * 64], src(k_d))
                nc.gpsimd.dma_start(qs[:, :, h * 64 : (h + 1) * 64], src(q_d))
                nc.gpsimd.dma_start(vs[:, :, h * 64 : (h + 1) * 64], src(v_d))
            st.qt2 = tpose.tile([128, L], BF16, tag="qt", name="qt2")
            st.kt2 = tpose.tile([128, L], BF16, tag="kt", name="kt2")
            st.vo = vpool.tile([128, NT, 2, 68], BF16, tag="vo", name="vo")
            st.os_h = [
                outp.tile([128, NT, D], F32, tag=f"os{h}", name=f"os{h}")
                for h in range(2)
            ]
            for t in range(NT):
                nc.sync.dma_start(
                    st.kt2[:, t * 128 : (t + 1) * 128], ks[:, t, :], transpose=True
                )
                nc.sync.dma_start(
                    st.qt2[:, t * 128 : (t + 1) * 128], qs[:, t, :], transpose=True
                )
            for h in range(2):
                nc.vector.tensor_copy(
                    st.vo[:, :, h, 0:64], vs[:, :, h * 64 : (h + 1) * 64]
                )
            nc.vector.memset(st.vo[:, :, :, 64:65], 1.0)

        def flush(pend):
            """Emit the AV matmuls for one pending unit; allocate the q-chunk
            accumulators on its first unit, normalize after its last."""
            st, qc, kt, pt = pend
            if kt == 0:
                st.accs = [
                    acc_.tile([128, 512], F32, tag=f"o{h}", name=f"o{h}")
                    for h in range(2)
                ]
            for h in range(2):
                for j in range(4):
                    # start=True clears has_written for the WHOLE bank: only
                    # the first matmul into each accumulator bank may set it.
                    nc.tensor.matmul(
                        st.accs[h][:, j * 66 : j * 66 + 65],
                        pt[:, (h * 4 + j) * 128 : (h * 4 + j + 1) * 128],
                        st.vo[:, kt, h, 0:65],
                        start=(kt == 0 and j == 0),
                        stop=(kt == NT - 1),
                    )
            if kt == NT - 1:
                for h in range(2):
                    av = st.accs[h][:, 0:264].rearrange("q (j c) -> q j c", c=66)
                    rc = rcp.tile([128, 4, 1], F32, tag="rc", name="rc")
                    nc.vector.reciprocal(rc, av[:, :, 64:65])
                    nc.vector.tensor_mul(
                        st.os_h[h][:, qc * 4 : (qc + 1) * 4, :],
                        av[:, :, 0:64],
                        rc.to_broadcast([128, 4, 64]),
                    )
                if qc == NQC - 1:
                    for h in range(2):
                        nc.sync.dma_start(
                            o_d[2 * st.p + h].rearrange("(t q) d -> q t d", q=128),
                            st.os_h[h],
                        )

        # ---- flat unit stream over (rep, pair, q-chunk, k-tile) ----
        steps = [Step(p) for _ in range(reps) for p in range(NP)]
        preproc(steps[0])
        pending = []
        held = None  # (st, qc, kt) of a pair's low unit awaiting its big ACT
        u = 0
        for i, st in enumerate(steps):
            for qc in range(NQC):
                for kt in range(NT):
                    slot = u % 3
                    for h in range(2):
                        nc.tensor.matmul(
                            ss[:, slot, h * 512 : (h + 1) * 512],
                            st.kt2[
                                h * 64 : (h + 1) * 64, kt * 128 : (kt + 1) * 128
                            ],
                            st.qt2[
                                h * 64 : (h + 1) * 64, qc * QC : (qc + 1) * QC
                            ],
                            start=True,
                            stop=True,
                        )
                    if pending:
                        flush(pending.pop(0))
                    if slot == 0:
                        held = (st, qc, kt)
                    elif slot == 1:
                        pt = ptbp.tile([128, 2048], BF16, tag="ptb", name="ptb")
                        nc.scalar.activation(
                            pt, ss[:, 0:2, :], AF.Exp, scale=SCALE
                        )
                        hst, hqc, hkt = held
                        pending.append((hst, hqc, hkt, pt[:, 0:1024]))
                        pending.append((st, qc, kt, pt[:, 1024:2048]))
                        held = None
                    else:
                        pt = ptsp.tile([128, 1024], BF16, tag="pts", name="pts")
                        nc.scalar.activation(
                            pt, ss[:, 2, :], AF.Exp, scale=SCALE
                        )
                        pending.append((st, qc, kt, pt))
                    u += 1
                # prefetch the whole next step's preprocessing early in this
                # step's main loop (after its first q-chunk has started)
                if qc == 0 and i + 1 < len(steps):
                    preproc(steps[i + 1])
        if held is not None:  # trailing unit without a partner (u%3==1 never hit)
            st_, qc_, kt_ = held
            pt = ptsp.tile([128, 1024], BF16, tag="pts", name="pts")
            nc.scalar.activation(pt, ss[:, 0, :], AF.Exp, scale=SCALE)
            pending.append((st_, qc_, kt_, pt))
        while pending:
            flush(pending.pop(0))

    nc.compile()
    return nc


def _get_nc():
    global _NC_CACHE
    if _NC_CACHE is None:
        _NC_CACHE = _build_nc()
    return _NC_CACHE


def kernel(queries, keys, values):
    queries = np.ascontiguousarray(queries, dtype=np.float32)
    keys = np.ascontiguousarray(keys, dtype=np.float32)
    values = np.ascontiguousarray(values, dtype=np.float32)
    nc = _get_nc()
    in_maps = [
        {
            "queries": queries[c * H : (c + 1) * H],
            "keys": keys[c * H : (c + 1) * H],
            "values": values[c * H : (c + 1) * H],
        }
        for c in range(N_CORES)
    ]
    res = run_bass_kernel_spmd(nc, in_maps, core_ids=list(range(N_CORES)))
    return np.concatenate([r["out"] for r in res.results], axis=0)
